# revision 32
# baseline (speedup 1.0000x reference)
"""Trainium2 Bass kernel for 16-head self-attention (b=2, n=2048, dm=1024, dh=64).

Sharding (final): hybrid tensor-parallel -- K gathered, V replicated.
Each of 8 cores owns (batch g = c//4, sequence block r = c%4) and computes
Q, K and the output projection ONLY for its own 512 rows.  K^T slices are
exchanged within each batch group via two HBM AllGathers ([[0..3],[4..7]];
rank order == global key order), one per 4 head-pairs, issued right after
each half of the K projection.  V is projected over the FULL sequence on
every core from a second full-x input: collectives here cost ~8us fixed +
~45us/MB-in, queue serially, and their SDMA traffic starves concurrent
DMA, so gathering V too (tried) exposed ~100us of serial AllGather --
replicating V costs ~47us of PE that overlaps the K gathers instead.

DMA scheduling (the queues drain round-robin with no priority, ~2KB per
descriptor, ~22GB/s per queue): the critical 6MB (x^T own, Wk, xTf key
block 0, Wv) is loaded first; the remaining xTf blocks and Wq are emitted
after a K-staging DMA whose wait blocks the sync engine until K-proj
compute catches up, so their transfer starts ~20us in; Wo/bo load during
attention; the collectives are gated on the priority loads via a dummy
gpsimd read.

Attention (all bf16; 2.4e-3 frobenius rel err e2e, gate 2e-2): per head
pair hp the S^T matmul contracts the pair's full 128 K^T rows against
zero-padded Q^T (the other head's 64 lanes multiply zeros); O'' uses
[V_h | 1 | 0pad] as a 128-wide lhsT whose PSUM row 64 accumulates the
softmax denominator.  exp runs 1536 cols per ACT instruction (3-bank PSUM
tiles, 5x1536+1x512 per head): at 1024 wide the ~170ns inter-instruction
ACT overhead made ACT the pair pacer; at 1536 ACT (~8.1us/head) sits just
under PE (~8.4us/head).  The V_aug copy for head h+1 is emitted at the
START of head h so it does not queue on DVE behind the normalize chain.
1/denominator: op[64:96] is staged to SBUF and fed to
reciprocal_approx_fast (the custom DVE op misreads PSUM and NaNs on
1-partition slices; the accurate DVE reciprocal costs 3.35us/lane-row),
then gpsimd.partition_broadcast fans it across the 64 output partitions.
The output projection runs as a PSUM-accumulated tail (interleaving it
into the pairs was tried and lost: its DVE adds' SBUF traffic slowed
concurrent PE matmuls from 263ns to ~427ns).

History: replicated-KV f32r baseline 382us -> bf16 332us -> this design
317us.  Fully-sharded K+V with serial AllGathers measured 391us.
"""

import sys

for _p in ("/opt/trn_rl_repo", "/root/.axon_site/_ro/trn_rl_repo"):
    if _p not in sys.path:
        sys.path.append(_p)

import numpy as np

B = 2
N = 2048
DM = 1024
H = 16
DH = 64
INNER = H * DH  # 1024
NCORES = 8
RANKS = 4       # cores per batch group
QR = 512        # rows (queries == key slice) per core
SCALE = DH ** -0.5
GROUPS = [[0, 1, 2, 3], [4, 5, 6, 7]]

_cached = {}


def _build():
    import contextlib
    import concourse.bacc as bacc
    import concourse.tile as tile
    import concourse.mybir as mybir

    f32 = mybir.dt.float32
    bf16 = mybir.dt.bfloat16
    Exp = mybir.ActivationFunctionType.Exp

    nc = bacc.Bacc("TRN2", target_bir_lowering=False, debug=False,
                   enable_asserts=False)

    xT_d = nc.dram_tensor("xT", [DM, QR], bf16, kind="ExternalInput").ap()
    xTf_d = nc.dram_tensor("xTf", [DM, N], bf16, kind="ExternalInput").ap()
    Wq_d = nc.dram_tensor("Wq", [DM, INNER], bf16, kind="ExternalInput").ap()
    Wk_d = nc.dram_tensor("Wk", [DM, INNER], bf16, kind="ExternalInput").ap()
    Wv_d = nc.dram_tensor("Wv", [DM, INNER], bf16, kind="ExternalInput").ap()
    Wo_d = nc.dram_tensor("Wo", [INNER, DM], bf16, kind="ExternalInput").ap()
    bo_d = nc.dram_tensor("bo", [DM], f32, kind="ExternalInput").ap()
    out_d = nc.dram_tensor("out", [QR, DM], f32, kind="ExternalOutput").ap()

    A = DM // 128       # 8 contraction blocks
    IB = INNER // 128   # 8 inner blocks (== head pairs)
    KB = N // 128       # 16 key blocks (full sequence)
    QB = QR // 128      # 4 query blocks
    HKT = IB // 2 * 128  # rows per K-gather half (512)

    xT_m = xT_d.rearrange("(a p) q -> p a q", p=128)
    xTf_m = xTf_d.rearrange("(h a p) n -> h p a n", p=128, h=2)
    Wq_m = Wq_d.rearrange("(h a p) i -> h p a i", p=128, h=2)
    Wk_m = Wk_d.rearrange("(h a p) i -> h p a i", p=128, h=2)
    Wv_m = Wv_d.rearrange("(h a p) i -> h p a i", p=128, h=2)
    Wo_m = Wo_d.rearrange("(h a p) i -> h p a i", p=128, h=2)
    out_r = out_d.rearrange("(qb p) d -> qb p d", p=128)

    with tile.TileContext(nc) as tc, \
         nc.allow_low_precision(reason="bf16 matmul pipeline, validated e2e"), \
         contextlib.ExitStack() as ctx:
        persist = ctx.enter_context(tc.tile_pool(name="persist", bufs=1))
        QT_z = persist.tile([128, IB, 2, QR], bf16)
        OT_sb = persist.tile([128, IB, QR], bf16)      # O^T [inner, q]
        V_sb = persist.tile([128, KB, INNER], bf16)    # V, full sequence
        V_aug2 = persist.tile([128, 2, KB, 128], bf16)  # ping-pong [V|1|0]
        Wo_sb = persist.tile([128, IB, DM], bf16)
        bo_sb = persist.tile([128, DM], f32)
        onef = persist.tile([128, 1], f32)
        zerof = persist.tile([128, 1], f32)
        dummy = persist.tile([1, 8], f32)
        gate = persist.tile([128, 1], f32)

        dram = ctx.enter_context(
            tc.tile_pool(name="dram", bufs=1, space="DRAM"))
        KT_in = [dram.tile([HKT, QR], bf16, name=f"KT_in{i}")
                 for i in range(2)]
        KT_g = [dram.tile([RANKS * HKT, QR], bf16, name=f"KT_g{i}")
                for i in range(2)]

        # ---------------- projections ----------------
        with tc.tile_pool(name="pa_x", bufs=1) as pa_x, \
             tc.tile_pool(name="pa_w", bufs=1) as pa_w, \
             tc.tile_pool(name="p_kstg", bufs=4) as pkstg, \
             tc.tile_pool(name="ps_k", bufs=6, space="PSUM") as psk:
            xT_sb = pa_x.tile([128, A, QR], bf16)
            xTf_sb = pa_x.tile([128, A, N], bf16)
            Wk_sb = pa_w.tile([128, A, INNER], bf16)
            Wv_sb = pa_w.tile([128, A, INNER], bf16)
            Wq_sb = pa_w.tile([128, A, INNER], bf16)
            # few big DMAs, in consumption order: K proj inputs, V's, Q's.
            # Wo/bo load later (only the post-attention tail needs them).
            nc.sync.dma_start(out=xT_sb, in_=xT_m)
            for hh in range(2):
                nc.sync.dma_start(out=Wk_sb[:, hh * 4:(hh + 1) * 4, :],
                                  in_=Wk_m[hh])
            # xTf split by column block: V-proj kb 0..3 needs only block 0,
            # so the V projection starts as soon as the K projection ends
            # instead of waiting for the full 4MB
            nc.sync.dma_start(out=xTf_sb[:, 0:4, 0:512],
                              in_=xTf_m[0][:, :, 0:512])
            nc.sync.dma_start(out=xTf_sb[:, 4:8, 0:512],
                              in_=xTf_m[1][:, :, 0:512])
            for hh in range(2):
                nc.sync.dma_start(out=Wv_sb[:, hh * 4:(hh + 1) * 4, :],
                                  in_=Wv_m[hh])


            # --- K projection (own keys), half at a time -> 2 AllGathers
            # (a single 1MB AllGather overlapping the whole V projection
            # measured worse: its SDMA window inflates every concurrent
            # engine; two gathers split the contention) ---
            for half in range(2):
                for ib in range(half * IB // 2, (half + 1) * IB // 2):
                    kp = psk.tile([128, QR], f32, tag="kp")
                    for a in range(A):
                        nc.tensor.matmul(
                            out=kp,
                            lhsT=Wk_sb[:, a, ib * 128:(ib + 1) * 128],
                            rhs=xT_sb[:, a, :],
                            start=(a == 0), stop=(a == A - 1))
                    kstg = pkstg.tile([128, QR], bf16, tag="kstg")
                    nc.vector.tensor_copy(out=kstg, in_=kp)
                    nc.sync.dma_start(
                        out=KT_in[half][(ib % 4) * 128:(ib % 4 + 1) * 128,
                                        :],
                        in_=kstg)
                if half == 0:
                    # the DMA queues drain round-robin with no priority, so
                    # the low-priority input loads (xTf key-blocks 1-3, Wq)
                    # are emitted HERE: the preceding kstg DMA's wait blocks
                    # the sync engine until K-proj compute reaches it, so
                    # these trigger ~20us in, after the critical 6MB
                    # (xT/Wk/xTf-block0/Wv) has the queues to itself.
                    for cb in range(1, 4):
                        nc.sync.dma_start(
                            out=xTf_sb[:, 0:4, cb * 512:(cb + 1) * 512],
                            in_=xTf_m[0][:, :, cb * 512:(cb + 1) * 512])
                        nc.sync.dma_start(
                            out=xTf_sb[:, 4:8, cb * 512:(cb + 1) * 512],
                            in_=xTf_m[1][:, :, cb * 512:(cb + 1) * 512])
                    for hh2 in range(2):
                        nc.sync.dma_start(
                            out=Wq_sb[:, hh2 * 4:(hh2 + 1) * 4, :],
                            in_=Wq_m[hh2])
                    # gate the collectives behind the xTf block-0 load
                    # (AG SDMA traffic starves concurrent input DMAs; Wv
                    # lands ~10us later and can tolerate the contention)
                    nc.gpsimd.tensor_copy(out=gate,
                                          in_=xTf_sb[:, 7, 511:512])
                nc.gpsimd.collective_compute(
                    "AllGather", mybir.AluOpType.bypass,
                    replica_groups=GROUPS,
                    ins=[KT_in[half][:]], outs=[KT_g[half][:]])

            # one-time initialization, emitted AFTER the K projection so
            # these DVE copies don't delay the K staging copies (which
            # recycle the projection PSUM tiles); nothing reads QT_z or
            # V_aug2 until Q-proj / attention
            nc.vector.memset(onef, 1.0)
            nc.vector.memset(zerof, 0.0)
            # warm the ACT exp table set during the projection phase
            nc.scalar.activation(out=dummy, in_=onef[0:1, 0:1]
                                 .to_broadcast([1, 8]), func=Exp)
            nc.vector.tensor_copy(
                out=QT_z[:, :, :, :],
                in_=zerof.unsqueeze(1).unsqueeze(1).to_broadcast(
                    [128, IB, 2, QR]))
            nc.vector.tensor_copy(
                out=V_aug2[:, :, :, 64:65],
                in_=onef.unsqueeze(1).unsqueeze(1).to_broadcast(
                    [128, 2, KB, 1]))
            nc.vector.tensor_copy(
                out=V_aug2[:, :, :, 65:128],
                in_=zerof.unsqueeze(1).unsqueeze(1).to_broadcast(
                    [128, 2, KB, 63]))

            # --- V projection (FULL sequence) into V_sb ---
            for kb in range(KB):
                for ic in range(2):
                    vp = psk.tile([128, 512], f32, tag="kp")
                    for a in range(A):
                        nc.tensor.matmul(
                            out=vp,
                            lhsT=xTf_sb[:, a, kb * 128:(kb + 1) * 128],
                            rhs=Wv_sb[:, a, ic * 512:(ic + 1) * 512],
                            start=(a == 0), stop=(a == A - 1))
                    nc.vector.tensor_copy(
                        out=V_sb[:, kb, ic * 512:(ic + 1) * 512], in_=vp)

            # --- Q projection (own rows) ---
            for ib in range(IB):
                qp = psk.tile([128, QR], f32, tag="kp")
                for a in range(A):
                    nc.tensor.matmul(
                        out=qp,
                        lhsT=Wq_sb[:, a, ib * 128:(ib + 1) * 128],
                        rhs=xT_sb[:, a, :],
                        start=(a == 0), stop=(a == A - 1))
                nc.vector.tensor_copy(out=QT_z[0:64, ib, 0, :],
                                      in_=qp[0:64, :])
                nc.vector.tensor_copy(out=QT_z[64:128, ib, 1, :],
                                      in_=qp[64:128, :])

        # deferred tail inputs: load during attention
        for hh in range(2):
            nc.sync.dma_start(out=Wo_sb[:, hh * 4:(hh + 1) * 4, :],
                              in_=Wo_m[hh])
        nc.gpsimd.dma_start(
            out=bo_sb, in_=bo_d.unsqueeze(0).to_broadcast([128, DM]))

        # ---------------- attention ----------------
        # exp runs 1536 cols per ACT instruction (3-bank PSUM tiles): at
        # 1024-wide the ~170ns inter-instruction ACT overhead made ACT the
        # pair pacer (16 x 1284ns > PE's 64 x 263ns).
        with tc.tile_pool(name="p_kt", bufs=2) as pkt, \
             tc.tile_pool(name="p_es", bufs=4) as pes, \
             tc.tile_pool(name="p_sm", bufs=1) as psm, \
             tc.tile_pool(name="ps_s", bufs=2, space="PSUM") as ps_s, \
             tc.tile_pool(name="ps_op", bufs=2, space="PSUM") as ps_op:

            def vaug_copy(h):
                nc.vector.tensor_copy(
                    out=V_aug2[:, h % 2, :, 0:64],
                    in_=V_sb[:, :, h * 64:(h + 1) * 64])

            vaug_copy(0)
            for hp in range(IB):
                KT_pair = pkt.tile([128, N], bf16, tag="kt")
                g, hpl = hp // 4, hp % 4
                for r in range(RANKS):
                    nc.sync.dma_start(
                        out=KT_pair[:, r * QR:(r + 1) * QR],
                        in_=KT_g[g][r * HKT + hpl * 128:
                                    r * HKT + (hpl + 1) * 128, :])
                for hh in range(2):
                    h = hp * 2 + hh
                    if h + 1 < H:
                        vaug_copy(h + 1)
                    op = ps_op.tile([128, QR], f32, tag="o")
                    kb = 0
                    for nk in (3, 3, 3, 3, 3, 1):  # 16 kb in 1536/512 tiles
                        sp = ps_s.tile([128, 1536], f32, tag="s")
                        for j in range(nk):
                            nc.tensor.matmul(
                                out=sp[:, j * 512:(j + 1) * 512],
                                lhsT=KT_pair[:, (kb + j) * 128:
                                             (kb + j + 1) * 128],
                                rhs=QT_z[:, hp, hh, :],
                                start=True, stop=True)
                        expS = pes.tile([128, 1536], bf16, tag="es")
                        nc.scalar.activation(out=expS[:, 0:nk * 512],
                                             in_=sp[:, 0:nk * 512],
                                             func=Exp, scale=SCALE)
                        for j in range(nk):
                            nc.tensor.matmul(
                                out=op,
                                lhsT=V_aug2[:, h % 2, kb + j, :],
                                rhs=expS[:, j * 512:(j + 1) * 512],
                                start=(kb + j == 0),
                                stop=(kb + j == KB - 1))
                        kb += nk
                    # approx reciprocal of the denominator row: the custom
                    # DVE op misreads PSUM and NaNs on 1-partition slices,
                    # so stage op[64:96] to SBUF (row 0 = denom, rest junk)
                    # and run it on 32 aligned partitions; the accurate DVE
                    # reciprocal would cost 3.35us/lane-row.
                    dstg = psm.tile([32, QR], f32, tag="dstg", bufs=2)
                    nc.vector.tensor_copy(out=dstg, in_=op[64:96, :])
                    recip = psm.tile([32, QR], f32, tag="recip", bufs=2)
                    nc.vector.reciprocal_approx_fast(out=recip, in_=dstg)
                    rbs = psm.tile([64, QR], f32, tag="rbs", bufs=2)
                    nc.gpsimd.partition_broadcast(rbs, recip[0:1, :],
                                                  channels=64)
                    nc.vector.tensor_mul(
                        OT_sb[hh * 64:(hh + 1) * 64, hp, :],
                        op[0:64, :], rbs)

        # ---------------- output projection (PSUM-accumulated tail) ------
        with tc.tile_pool(name="p_ob", bufs=4) as pob, \
             tc.tile_pool(name="ps_oc", bufs=4, space="PSUM") as ps_oc:
            for dc in range(2):
                for qb in range(QB):
                    outp = ps_oc.tile([128, 512], f32, tag="oc")
                    for ib in range(IB):
                        nc.tensor.matmul(
                            out=outp,
                            lhsT=OT_sb[:, ib, qb * 128:(qb + 1) * 128],
                            rhs=Wo_sb[:, ib, dc * 512:(dc + 1) * 512],
                            start=(ib == 0), stop=(ib == IB - 1))
                    ob = pob.tile([128, 512], f32, tag="ob")
                    nc.vector.tensor_add(
                        ob, outp, bo_sb[:, dc * 512:(dc + 1) * 512])
                    nc.sync.dma_start(
                        out=out_r[qb, :, dc * 512:(dc + 1) * 512], in_=ob)

    nc.compile()
    return nc


def _get_nc():
    if "nc" not in _cached:
        _cached["nc"] = _build()
    return _cached["nc"]


def kernel(queries, Wq, Wkv, Wo, bo, _trace=False):
    import ml_dtypes
    from concourse.bass_utils import run_bass_kernel_spmd

    bf = ml_dtypes.bfloat16
    queries = np.asarray(queries, dtype=np.float32)
    Wq_c = np.asarray(Wq, dtype=np.float32).astype(bf)
    Wkv = np.asarray(Wkv, dtype=np.float32)
    Wk_c = np.ascontiguousarray(Wkv[:, :INNER]).astype(bf)
    Wv_c = np.ascontiguousarray(Wkv[:, INNER:]).astype(bf)
    Wo_c = np.asarray(Wo, dtype=np.float32).astype(bf)
    bo = np.asarray(bo, dtype=np.float32)

    nc = _get_nc()

    xTf = [np.ascontiguousarray(queries[g].T).astype(bf) for g in range(B)]
    in_maps = []
    for c in range(NCORES):
        g, r = c // RANKS, c % RANKS
        xT = np.ascontiguousarray(xTf[g][:, r * QR:(r + 1) * QR])
        in_maps.append({"xT": xT, "xTf": xTf[g], "Wq": Wq_c, "Wk": Wk_c,
                        "Wv": Wv_c, "Wo": Wo_c, "bo": bo})

    res = run_bass_kernel_spmd(nc, in_maps, list(range(NCORES)),
                               trace=_trace)
    out = np.empty((B, N, DM), dtype=np.float32)
    for c in range(NCORES):
        g, r = c // RANKS, c % RANKS
        out[g, r * QR:(r + 1) * QR, :] = res.results[c]["out"]
    if _trace:
        return out, res
    return out


if __name__ == "__main__":
    rng = np.random.default_rng(0)
    s = 0.02
    inputs = dict(
        queries=rng.standard_normal((B, N, DM), dtype=np.float32),
        Wq=(rng.standard_normal((DM, INNER), dtype=np.float32) * s),
        Wkv=(rng.standard_normal((DM, 2 * INNER), dtype=np.float32) * s),
        Wo=(rng.standard_normal((INNER, DM), dtype=np.float32) * s),
        bo=(rng.standard_normal((DM,), dtype=np.float32) * s),
    )
    out = kernel(**inputs)
    print("kernel ran, out shape", out.shape)


# revision 33
# speedup vs baseline: 1.0115x; 1.0115x over previous
"""Trainium2 Bass kernel for 16-head self-attention (b=2, n=2048, dm=1024, dh=64).

Sharding (final): hybrid tensor-parallel -- K gathered, V replicated.
Each of 8 cores owns (batch g = c//4, sequence block r = c%4) and computes
Q, K and the output projection ONLY for its own 512 rows.  K^T slices are
exchanged within each batch group via two HBM AllGathers ([[0..3],[4..7]];
rank order == global key order), one per 4 head-pairs, issued right after
each half of the K projection.  V is projected over the FULL sequence on
every core from a second full-x input: collectives here cost ~8us fixed +
~45us/MB-in, queue serially, and their SDMA traffic starves concurrent
DMA, so gathering V too (tried) exposed ~100us of serial AllGather --
replicating V costs ~47us of PE that overlaps the K gathers instead.

DMA scheduling (the queues drain round-robin with no priority, ~2KB per
descriptor, ~22GB/s per queue): the critical 6MB (x^T own, Wk, xTf key
block 0, Wv) is loaded first; the remaining xTf blocks and Wq are emitted
after a K-staging DMA whose wait blocks the sync engine until K-proj
compute catches up, so their transfer starts ~20us in; Wo/bo load during
attention; the collectives are gated on the priority loads via a dummy
gpsimd read.

Attention (all bf16; 2.4e-3 frobenius rel err e2e, gate 2e-2): per head
pair hp the S^T matmul contracts the pair's full 128 K^T rows against
zero-padded Q^T (the other head's 64 lanes multiply zeros); O'' uses
[V_h | 1 | 0pad] as a 128-wide lhsT whose PSUM row 64 accumulates the
softmax denominator.  exp runs 1536 cols per ACT instruction (3-bank PSUM
tiles, 5x1536+1x512 per head): at 1024 wide the ~170ns inter-instruction
ACT overhead made ACT the pair pacer; at 1536 ACT (~8.1us/head) sits just
under PE (~8.4us/head).  The V_aug copy for head h+1 is emitted at the
START of head h so it does not queue on DVE behind the normalize chain.
1/denominator: op[64:96] is staged to SBUF and fed to
reciprocal_approx_fast (the custom DVE op misreads PSUM and NaNs on
1-partition slices; the accurate DVE reciprocal costs 3.35us/lane-row),
then gpsimd.partition_broadcast fans it across the 64 output partitions.
The output projection runs as a PSUM-accumulated tail (interleaving it
into the pairs was tried and lost: its DVE adds' SBUF traffic slowed
concurrent PE matmuls from 263ns to ~427ns).

History: replicated-KV f32r baseline 382us -> bf16 332us -> this design
317us.  Fully-sharded K+V with serial AllGathers measured 391us.
"""

import sys

for _p in ("/opt/trn_rl_repo", "/root/.axon_site/_ro/trn_rl_repo"):
    if _p not in sys.path:
        sys.path.append(_p)

import numpy as np

B = 2
N = 2048
DM = 1024
H = 16
DH = 64
INNER = H * DH  # 1024
NCORES = 8
RANKS = 4       # cores per batch group
QR = 512        # rows (queries == key slice) per core
SCALE = DH ** -0.5
GROUPS = [[0, 1, 2, 3], [4, 5, 6, 7]]

_cached = {}


def _build():
    import contextlib
    import concourse.bacc as bacc
    import concourse.tile as tile
    import concourse.mybir as mybir

    f32 = mybir.dt.float32
    bf16 = mybir.dt.bfloat16
    Exp = mybir.ActivationFunctionType.Exp

    nc = bacc.Bacc("TRN2", target_bir_lowering=False, debug=False,
                   enable_asserts=False)

    xT_d = nc.dram_tensor("xT", [DM, QR], bf16, kind="ExternalInput").ap()
    xTf_d = nc.dram_tensor("xTf", [DM, N], bf16, kind="ExternalInput").ap()
    Wq_d = nc.dram_tensor("Wq", [DM, INNER], bf16, kind="ExternalInput").ap()
    Wk_d = nc.dram_tensor("Wk", [DM, INNER], bf16, kind="ExternalInput").ap()
    Wv_d = nc.dram_tensor("Wv", [DM, INNER], bf16, kind="ExternalInput").ap()
    Wo_d = nc.dram_tensor("Wo", [INNER, DM], bf16, kind="ExternalInput").ap()
    bo_d = nc.dram_tensor("bo", [DM], f32, kind="ExternalInput").ap()
    out_d = nc.dram_tensor("out", [QR, DM], f32, kind="ExternalOutput").ap()

    A = DM // 128       # 8 contraction blocks
    IB = INNER // 128   # 8 inner blocks (== head pairs)
    KB = N // 128       # 16 key blocks (full sequence)
    QB = QR // 128      # 4 query blocks
    HKT = IB // 2 * 128  # rows per K-gather half (512)

    xT_m = xT_d.rearrange("(a p) q -> p a q", p=128)
    xTf_m = xTf_d.rearrange("(h a p) n -> h p a n", p=128, h=2)
    Wq_m = Wq_d.rearrange("(h a p) i -> h p a i", p=128, h=2)
    Wk_m = Wk_d.rearrange("(h a p) i -> h p a i", p=128, h=2)
    Wv_m = Wv_d.rearrange("(h a p) i -> h p a i", p=128, h=2)
    Wo_m = Wo_d.rearrange("(h a p) i -> h p a i", p=128, h=2)
    out_r = out_d.rearrange("(qb p) d -> qb p d", p=128)

    with tile.TileContext(nc) as tc, \
         nc.allow_low_precision(reason="bf16 matmul pipeline, validated e2e"), \
         contextlib.ExitStack() as ctx:
        persist = ctx.enter_context(tc.tile_pool(name="persist", bufs=1))
        QT_z = persist.tile([128, IB, 2, QR], bf16)
        OT_sb = persist.tile([128, IB, QR], bf16)      # O^T [inner, q]
        V_sb = persist.tile([128, KB, INNER], bf16)    # V, full sequence
        V_aug2 = persist.tile([128, 2, KB, 128], bf16)  # ping-pong [V|1|0]
        Wo_sb = persist.tile([128, IB, DM], bf16)
        bo_sb = persist.tile([128, DM], f32)
        onef = persist.tile([128, 1], f32)
        zerof = persist.tile([128, 1], f32)
        dummy = persist.tile([1, 8], f32)
        gate = persist.tile([128, 1], f32)

        dram = ctx.enter_context(
            tc.tile_pool(name="dram", bufs=1, space="DRAM"))
        KT_in = [dram.tile([HKT, QR], bf16, name=f"KT_in{i}")
                 for i in range(2)]
        KT_g = [dram.tile([RANKS * HKT, QR], bf16, name=f"KT_g{i}")
                for i in range(2)]

        # ---------------- projections ----------------
        with tc.tile_pool(name="pa_x", bufs=1) as pa_x, \
             tc.tile_pool(name="pa_w", bufs=1) as pa_w, \
             tc.tile_pool(name="p_kstg", bufs=4) as pkstg, \
             tc.tile_pool(name="ps_k", bufs=4, space="PSUM") as psk:
            xT_sb = pa_x.tile([128, A, QR], bf16)
            xTf_sb = pa_x.tile([128, A, N], bf16)
            Wk_sb = pa_w.tile([128, A, INNER], bf16)
            Wv_sb = pa_w.tile([128, A, INNER], bf16)
            Wq_sb = pa_w.tile([128, A, INNER], bf16)
            # few big DMAs, in consumption order: K proj inputs, V's, Q's.
            # Wo/bo load later (only the post-attention tail needs them).
            nc.sync.dma_start(out=xT_sb, in_=xT_m)
            for hh in range(2):
                nc.sync.dma_start(out=Wk_sb[:, hh * 4:(hh + 1) * 4, :],
                                  in_=Wk_m[hh])
            # xTf split by column block: V-proj kb 0..3 needs only block 0,
            # so the V projection starts as soon as the K projection ends
            # instead of waiting for the full 4MB
            nc.sync.dma_start(out=xTf_sb[:, 0:4, 0:512],
                              in_=xTf_m[0][:, :, 0:512])
            nc.sync.dma_start(out=xTf_sb[:, 4:8, 0:512],
                              in_=xTf_m[1][:, :, 0:512])
            for hh in range(2):
                nc.sync.dma_start(out=Wv_sb[:, hh * 4:(hh + 1) * 4, :],
                                  in_=Wv_m[hh])

            # constants / one-time initialization
            nc.vector.memset(onef, 1.0)
            nc.vector.memset(zerof, 0.0)
            # warm the ACT exp table set during the projection phase
            nc.scalar.activation(out=dummy, in_=onef[0:1, 0:1]
                                 .to_broadcast([1, 8]), func=Exp)
            nc.vector.tensor_copy(
                out=QT_z[:, :, :, :],
                in_=zerof.unsqueeze(1).unsqueeze(1).to_broadcast(
                    [128, IB, 2, QR]))
            nc.vector.tensor_copy(
                out=V_aug2[:, :, :, 64:65],
                in_=onef.unsqueeze(1).unsqueeze(1).to_broadcast(
                    [128, 2, KB, 1]))
            nc.vector.tensor_copy(
                out=V_aug2[:, :, :, 65:128],
                in_=zerof.unsqueeze(1).unsqueeze(1).to_broadcast(
                    [128, 2, KB, 63]))


            # --- K projection (own keys), half at a time -> 2 AllGathers
            # (a single 1MB AllGather overlapping the whole V projection
            # measured worse: its SDMA window inflates every concurrent
            # engine; two gathers split the contention) ---
            for half in range(2):
                for ib in range(half * IB // 2, (half + 1) * IB // 2):
                    kp = psk.tile([128, QR], f32, tag="kp")
                    for a in range(A):
                        nc.tensor.matmul(
                            out=kp,
                            lhsT=Wk_sb[:, a, ib * 128:(ib + 1) * 128],
                            rhs=xT_sb[:, a, :],
                            start=(a == 0), stop=(a == A - 1))
                    kstg = pkstg.tile([128, QR], bf16, tag="kstg")
                    nc.vector.tensor_copy(out=kstg, in_=kp)
                    nc.sync.dma_start(
                        out=KT_in[half][(ib % 4) * 128:(ib % 4 + 1) * 128,
                                        :],
                        in_=kstg)
                if half == 0:
                    # the DMA queues drain round-robin with no priority, so
                    # the low-priority input loads (xTf key-blocks 1-3, Wq)
                    # are emitted HERE: the preceding kstg DMA's wait blocks
                    # the sync engine until K-proj compute reaches it, so
                    # these trigger ~20us in, after the critical 6MB
                    # (xT/Wk/xTf-block0/Wv) has the queues to itself.
                    for cb in range(1, 4):
                        nc.sync.dma_start(
                            out=xTf_sb[:, 0:4, cb * 512:(cb + 1) * 512],
                            in_=xTf_m[0][:, :, cb * 512:(cb + 1) * 512])
                        nc.sync.dma_start(
                            out=xTf_sb[:, 4:8, cb * 512:(cb + 1) * 512],
                            in_=xTf_m[1][:, :, cb * 512:(cb + 1) * 512])
                    for hh2 in range(2):
                        nc.sync.dma_start(
                            out=Wq_sb[:, hh2 * 4:(hh2 + 1) * 4, :],
                            in_=Wq_m[hh2])
                    # gate the collectives behind the priority inputs: AG
                    # SDMA traffic starves concurrent input DMAs
                    nc.gpsimd.tensor_copy(out=gate,
                                          in_=Wv_sb[:, 7, 1023:1024])
                nc.gpsimd.collective_compute(
                    "AllGather", mybir.AluOpType.bypass,
                    replica_groups=GROUPS,
                    ins=[KT_in[half][:]], outs=[KT_g[half][:]])

            # --- V projection (FULL sequence) into V_sb ---
            for kb in range(KB):
                for ic in range(2):
                    vp = psk.tile([128, 512], f32, tag="kp")
                    for a in range(A):
                        nc.tensor.matmul(
                            out=vp,
                            lhsT=xTf_sb[:, a, kb * 128:(kb + 1) * 128],
                            rhs=Wv_sb[:, a, ic * 512:(ic + 1) * 512],
                            start=(a == 0), stop=(a == A - 1))
                    nc.vector.tensor_copy(
                        out=V_sb[:, kb, ic * 512:(ic + 1) * 512], in_=vp)

            # --- Q projection (own rows) ---
            for ib in range(IB):
                qp = psk.tile([128, QR], f32, tag="kp")
                for a in range(A):
                    nc.tensor.matmul(
                        out=qp,
                        lhsT=Wq_sb[:, a, ib * 128:(ib + 1) * 128],
                        rhs=xT_sb[:, a, :],
                        start=(a == 0), stop=(a == A - 1))
                nc.vector.tensor_copy(out=QT_z[0:64, ib, 0, :],
                                      in_=qp[0:64, :])
                nc.vector.tensor_copy(out=QT_z[64:128, ib, 1, :],
                                      in_=qp[64:128, :])

        # deferred tail inputs: load during attention
        for hh in range(2):
            nc.sync.dma_start(out=Wo_sb[:, hh * 4:(hh + 1) * 4, :],
                              in_=Wo_m[hh])
        nc.gpsimd.dma_start(
            out=bo_sb, in_=bo_d.unsqueeze(0).to_broadcast([128, DM]))

        # ---------------- attention ----------------
        # exp runs 1536 cols per ACT instruction (3-bank PSUM tiles): at
        # 1024-wide the ~170ns inter-instruction ACT overhead made ACT the
        # pair pacer (16 x 1284ns > PE's 64 x 263ns).
        with tc.tile_pool(name="p_kt", bufs=2) as pkt, \
             tc.tile_pool(name="p_es", bufs=4) as pes, \
             tc.tile_pool(name="p_sm", bufs=1) as psm, \
             tc.tile_pool(name="ps_s", bufs=2, space="PSUM") as ps_s, \
             tc.tile_pool(name="ps_op", bufs=2, space="PSUM") as ps_op:

            def vaug_copy(h):
                nc.vector.tensor_copy(
                    out=V_aug2[:, h % 2, :, 0:64],
                    in_=V_sb[:, :, h * 64:(h + 1) * 64])

            vaug_copy(0)
            for hp in range(IB):
                KT_pair = pkt.tile([128, N], bf16, tag="kt")
                g, hpl = hp // 4, hp % 4
                for r in range(RANKS):
                    nc.sync.dma_start(
                        out=KT_pair[:, r * QR:(r + 1) * QR],
                        in_=KT_g[g][r * HKT + hpl * 128:
                                    r * HKT + (hpl + 1) * 128, :])
                for hh in range(2):
                    h = hp * 2 + hh
                    if h + 1 < H:
                        vaug_copy(h + 1)
                    op = ps_op.tile([128, QR], f32, tag="o")
                    kb = 0
                    for nk in (3, 3, 3, 3, 3, 1):  # 16 kb in 1536/512 tiles
                        sp = ps_s.tile([128, 1536], f32, tag="s")
                        for j in range(nk):
                            nc.tensor.matmul(
                                out=sp[:, j * 512:(j + 1) * 512],
                                lhsT=KT_pair[:, (kb + j) * 128:
                                             (kb + j + 1) * 128],
                                rhs=QT_z[:, hp, hh, :],
                                start=True, stop=True)
                        expS = pes.tile([128, 1536], bf16, tag="es")
                        nc.scalar.activation(out=expS[:, 0:nk * 512],
                                             in_=sp[:, 0:nk * 512],
                                             func=Exp, scale=SCALE)
                        for j in range(nk):
                            nc.tensor.matmul(
                                out=op,
                                lhsT=V_aug2[:, h % 2, kb + j, :],
                                rhs=expS[:, j * 512:(j + 1) * 512],
                                start=(kb + j == 0),
                                stop=(kb + j == KB - 1))
                        kb += nk
                    # approx reciprocal of the denominator row: the custom
                    # DVE op misreads PSUM and NaNs on 1-partition slices,
                    # so stage op[64:96] to SBUF (row 0 = denom, rest junk)
                    # and run it on 32 aligned partitions; the accurate DVE
                    # reciprocal would cost 3.35us/lane-row.
                    dstg = psm.tile([32, QR], f32, tag="dstg", bufs=2)
                    nc.vector.tensor_copy(out=dstg, in_=op[64:96, :])
                    recip = psm.tile([32, QR], f32, tag="recip", bufs=2)
                    nc.vector.reciprocal_approx_fast(out=recip, in_=dstg)
                    rbs = psm.tile([64, QR], f32, tag="rbs", bufs=2)
                    nc.gpsimd.partition_broadcast(rbs, recip[0:1, :],
                                                  channels=64)
                    nc.vector.tensor_mul(
                        OT_sb[hh * 64:(hh + 1) * 64, hp, :],
                        op[0:64, :], rbs)

        # ---------------- output projection (PSUM-accumulated tail) ------
        with tc.tile_pool(name="p_ob", bufs=4) as pob, \
             tc.tile_pool(name="ps_oc", bufs=4, space="PSUM") as ps_oc:
            for dc in range(2):
                for qb in range(QB):
                    outp = ps_oc.tile([128, 512], f32, tag="oc")
                    for ib in range(IB):
                        nc.tensor.matmul(
                            out=outp,
                            lhsT=OT_sb[:, ib, qb * 128:(qb + 1) * 128],
                            rhs=Wo_sb[:, ib, dc * 512:(dc + 1) * 512],
                            start=(ib == 0), stop=(ib == IB - 1))
                    ob = pob.tile([128, 512], f32, tag="ob")
                    nc.vector.tensor_add(
                        ob, outp, bo_sb[:, dc * 512:(dc + 1) * 512])
                    nc.sync.dma_start(
                        out=out_r[qb, :, dc * 512:(dc + 1) * 512], in_=ob)

    nc.compile()
    return nc


def _get_nc():
    if "nc" not in _cached:
        _cached["nc"] = _build()
    return _cached["nc"]


def kernel(queries, Wq, Wkv, Wo, bo, _trace=False):
    import ml_dtypes
    from concourse.bass_utils import run_bass_kernel_spmd

    bf = ml_dtypes.bfloat16
    queries = np.asarray(queries, dtype=np.float32)
    Wq_c = np.asarray(Wq, dtype=np.float32).astype(bf)
    Wkv = np.asarray(Wkv, dtype=np.float32)
    Wk_c = np.ascontiguousarray(Wkv[:, :INNER]).astype(bf)
    Wv_c = np.ascontiguousarray(Wkv[:, INNER:]).astype(bf)
    Wo_c = np.asarray(Wo, dtype=np.float32).astype(bf)
    bo = np.asarray(bo, dtype=np.float32)

    nc = _get_nc()

    xTf = [np.ascontiguousarray(queries[g].T).astype(bf) for g in range(B)]
    in_maps = []
    for c in range(NCORES):
        g, r = c // RANKS, c % RANKS
        xT = np.ascontiguousarray(xTf[g][:, r * QR:(r + 1) * QR])
        in_maps.append({"xT": xT, "xTf": xTf[g], "Wq": Wq_c, "Wk": Wk_c,
                        "Wv": Wv_c, "Wo": Wo_c, "bo": bo})

    res = run_bass_kernel_spmd(nc, in_maps, list(range(NCORES)),
                               trace=_trace)
    out = np.empty((B, N, DM), dtype=np.float32)
    for c in range(NCORES):
        g, r = c // RANKS, c % RANKS
        out[g, r * QR:(r + 1) * QR, :] = res.results[c]["out"]
    if _trace:
        return out, res
    return out


if __name__ == "__main__":
    rng = np.random.default_rng(0)
    s = 0.02
    inputs = dict(
        queries=rng.standard_normal((B, N, DM), dtype=np.float32),
        Wq=(rng.standard_normal((DM, INNER), dtype=np.float32) * s),
        Wkv=(rng.standard_normal((DM, 2 * INNER), dtype=np.float32) * s),
        Wo=(rng.standard_normal((INNER, DM), dtype=np.float32) * s),
        bo=(rng.standard_normal((DM,), dtype=np.float32) * s),
    )
    out = kernel(**inputs)
    print("kernel ran, out shape", out.shape)
